# revision 35
# baseline (speedup 1.0000x reference)
"""3-layer GCN (ContrastiveGNN) on 8 Trainium2 NeuronCores.

Strategy (dst-sharded edge partition, "1D graph partition"):
  - Nodes are split into 8 blocks of 12500 dsts; device i owns block i and all
    edges whose dst lands in its block.  Self-loops are NOT gathered: their
    contribution dis[d]^2*h[d] is added locally from the previous layer's
    stage tile (layer 1 uses a per-device staged copy of dis*x).
  - Math reorder: for each GCN layer,
        out = D^-1/2 (A+I) D^-1/2 (h W) + b  ==  dis_d * (sum_{e->d} T[src]) @ W + b
    with T = dis * h (row-scaled activations).  Aggregation happens BEFORE the
    dense transform, so the gather tables carry 128 features for every layer.
  - Aggregation on the tensor engine: edges are DENSELY packed per
    (window-batch, src-region) call, sorted by dst-window; each 128-slot
    gather group feeds one_hot[slot, dst_local].T @ gathered[slot, feat]
    matmuls accumulated in PSUM per 128-dst window.  A group may straddle
    window boundaries: for each (group, window) pair that is non-empty on ANY
    device there is one static matmul + one one-hot column; per-device
    one-hot entries are -1 (-> all-zero column) for slots outside the window,
    so the SPMD-uniform instruction stream stays correct per device.
  - Gathers use the SWDGE dma_gather custom instruction (int16 indices =>
    the 100352-row table is addressed in 4 chunk regions of <=32768 rows).
  - Tables are bf16 (PSUM accumulation f32); between layers the per-device
    table blocks are exchanged with chunked AllGather collectives overlapped
    with the remaining windows' compute.
  - All 8 devices run one SPMD program: per-call slot capacities and
    (group, window) matmul lists are maxed/unioned over devices, so
    instruction streams are identical and only input data differs.
"""

import numpy as np
import ml_dtypes

BF16 = ml_dtypes.bfloat16

N = 100000
F = 128
DOUT = 64
M = 8
BLK = N // M            # 12500 dst nodes per device
P = 128
WPD = (BLK + P - 1) // P  # 98 windows per device
BLKP = WPD * P            # 12544 padded block rows
TROWS = M * BLKP          # 100352 table rows
NPAIR = 4
import os as _os
WB = int(_os.environ.get("GNN_WB", "6"))  # windows per gather batch
NBATCH = (WPD + WB - 1) // WB

# AG sub-chunks (staging/collective units) and gather regions (int16
# addressing units).  Sub-chunk s covers batches SUBB[s]..SUBB[s+1] and
# belongs to region REGOF[s]; regions are the 4 contiguous table areas a
# gather call can address; within a region rows are sub-chunk-major.
SUBB = [int(b) for b in _os.environ.get("GNN_SUBB", "0,5,10,15,17" if WB == 6 else "").split(",")]
NSUB = len(SUBB) - 1
REGOF = [min(s, 3) for s in range(NSUB)]        # subs >=3 share region 3
SUBW = [min(SUBB[s + 1] * WB, WPD) - SUBB[s] * WB for s in range(NSUB)]
SUBBASE = [SUBB[s] * WB * P for s in range(NSUB)]  # local row base of sub
SUBROWS = [w * P for w in SUBW]                 # local rows per sub
# region-local row base of each sub (sub-chunk-major within region)
SUBREGB = []
_acc = {}
for _s in range(NSUB):
    _r = REGOF[_s]
    SUBREGB.append(_acc.get(_r, 0))
    _acc[_r] = _acc.get(_r, 0) + M * SUBROWS[_s]
REGR = [_acc.get(r, 0) for r in range(4)]       # table rows per region
REGB = [0, 0, 0, 0]
for _r in range(1, 4):
    REGB[_r] = REGB[_r - 1] + REGR[_r - 1]      # table row base of region
assert all(r <= 32768 for r in REGR), REGR


class _Call:
    __slots__ = ("ic0", "c16", "dc0", "ncols", "c128", "nslots", "mms")


def _preprocess(x, edge_index, W1, b1, W2, b2, W3, b3):
    """Host-side index plumbing + input staging. Returns (meta, per-core
    in_maps)."""
    x = np.asarray(x, np.float32)
    ei = np.asarray(edge_index)
    src = ei[0].astype(np.int64)
    dst = ei[1].astype(np.int64)

    # degree/dis INCLUDE self-loops (reference adds them), but self-loop
    # edges are handled locally, not gathered.
    deg = (np.bincount(dst, minlength=N) + 1).astype(np.float32)
    dis = (1.0 / np.sqrt(deg)).astype(np.float32)

    # layer-1 gather table: dis-scaled input features, zeroed pad rows,
    # sub-chunk-major layout: sub s = [M blocks x SUBROWS[s] local rows]
    xs = x * dis[:, None]
    T1 = np.zeros((TROWS, F), BF16)
    for s in range(NSUB):
        for j in range(M):
            lo = j * BLK + SUBBASE[s]
            hi = min(j * BLK + SUBBASE[s] + SUBROWS[s], (j + 1) * BLK)
            db = REGB[REGOF[s]] + SUBREGB[s] + j * SUBROWS[s]
            T1[db : db + (hi - lo)] = xs[lo:hi].astype(BF16)

    dev = dst // BLK
    j_src = src // BLK
    loc_src = src - j_src * BLK
    # src sub-chunk id -> region + region-local row
    s_src = np.searchsorted(np.array(SUBBASE[1:], np.int64), loc_src, side="right")
    subrows = np.array(SUBROWS, np.int64)
    subbase = np.array(SUBBASE, np.int64)
    subregb = np.array(SUBREGB, np.int64)
    regof = np.array(REGOF, np.int64)
    c_src = regof[s_src]
    rel = (
        subregb[s_src] + j_src * subrows[s_src] + (loc_src - subbase[s_src])
    ).astype(np.int64)
    dloc = dst - dev * BLK
    w_arr = dloc // P
    dwin = (dloc - w_arr * P).astype(np.int64)
    wb_arr = w_arr // WB
    # bucket key: (batch, region, window) — buckets of one call are contiguous
    bkey = (wb_arr * NPAIR + c_src) * WPD + w_arr
    NBUCK = NBATCH * NPAIR * WPD

    cnt = np.zeros((M, NBUCK), np.int64)
    for i in range(M):
        cnt[i] = np.bincount(bkey[dev == i], minlength=NBUCK)

    # window start/end slot positions per device within each call
    # (dense packing: edges of a call sorted by window, tightly packed)
    meta_calls = {}
    gtot = np.zeros(WPD, np.int64)
    ic = dc = 0
    for wb in range(NBATCH):
        w0 = wb * WB
        wcnt = min(WB, WPD - w0)
        for p in range(NPAIR):
            c = _Call()
            c.ic0, c.dc0 = ic, dc
            bids = [(wb * NPAIR + p) * WPD + w for w in range(w0, w0 + wcnt)]
            ccnt = cnt[:, bids]                       # [M, wcnt]
            ends = np.cumsum(ccnt, axis=1)            # [M, wcnt]
            starts = ends - ccnt
            nmax = int(ends[:, -1].max())
            nslots = -(-max(nmax, 1) // P) * P
            c.nslots = nslots
            c.c16 = nslots // 16
            c.c128 = nslots // P
            c.mms = {}
            ncols = 0
            for k, w in enumerate(range(w0, w0 + wcnt)):
                has = ccnt[:, k] > 0
                if not has.any():
                    c.mms[w] = []
                    continue
                g_lo = int(starts[has, k].min()) // P
                g_hi = -(-int(ends[has, k].max()) // P)
                c.mms[w] = [(g, ncols + j) for j, g in enumerate(range(g_lo, g_hi))]
                ncols += g_hi - g_lo
                gtot[w] += g_hi - g_lo
            c.ncols = ncols
            ic += c.c16
            dc += c.ncols
            meta_calls[(wb, p)] = c
    sc16, sccols = ic, dc

    meta = {"calls": meta_calls, "gtot": gtot, "sc16": sc16, "sccols": sccols}

    iota_np = np.tile(np.arange(P, dtype=np.float32).astype(BF16), (P, 1)).reshape(
        P, 1, P
    )
    ident_np = np.eye(P, dtype=np.float32).astype(BF16)
    w1b = np.asarray(W1, np.float32).astype(BF16)
    w2b = np.asarray(W2, np.float32).astype(BF16)
    w3b = np.asarray(W3, np.float32).astype(BF16)
    b1f = np.tile(np.asarray(b1, np.float32), (P, 1))
    b2f = np.tile(np.asarray(b2, np.float32), (P, 1))
    b3f = np.tile(np.asarray(b3, np.float32), (P, 1))

    in_maps = []
    for i in range(M):
        m = dev == i
        bk = bkey[m]
        o = np.argsort(bk, kind="stable")
        bk_s = bk[o]
        rel_s = rel[m][o].astype(np.int16)
        dw_s = dwin[m][o]

        i16_parts, dl_parts = [], []
        pos = 0
        for wb in range(NBATCH):
            w0 = wb * WB
            wcnt = min(WB, WPD - w0)
            for p in range(NPAIR):
                c = meta_calls[(wb, p)]
                bids = [(wb * NPAIR + p) * WPD + w for w in range(w0, w0 + wcnt)]
                ccnt_i = cnt[i][bids]
                n_i = int(ccnt_i.sum())
                seg_r = rel_s[pos : pos + n_i]
                seg_w = dw_s[pos : pos + n_i]
                pos += n_i
                idxfl = np.zeros(c.nslots, np.int16)
                idxfl[:n_i] = seg_r
                dl = np.full((P, c.ncols), -1.0, np.float32)
                ends_i = np.cumsum(ccnt_i)
                starts_i = ends_i - ccnt_i
                for k, w in enumerate(range(w0, w0 + wcnt)):
                    st, en = int(starts_i[k]), int(ends_i[k])
                    if st == en:
                        continue
                    for g, col in c.mms[w]:
                        lo = max(st, g * P)
                        hi = min(en, (g + 1) * P)
                        if lo < hi:
                            dl[lo - g * P : hi - g * P, col] = seg_w[lo:hi]
                i16_parts.append(idxfl.reshape(-1, 16).T)
                dl_parts.append(dl)
        idx16 = np.tile(np.concatenate(i16_parts, axis=1), (8, 1))
        dl128 = np.concatenate(dl_parts, axis=1).astype(BF16)

        disp = np.zeros(BLKP, np.float32)
        disp[:BLK] = dis[i * BLK : (i + 1) * BLK]
        disb = disp.reshape(WPD, P).T.copy()

        # own-block layer-1 table rows in stage layout [P, WPD*F]
        xsp = np.zeros((BLKP, F), np.float32)
        xsp[:BLK] = xs[i * BLK : (i + 1) * BLK]
        xso = (
            xsp.reshape(WPD, P, F).transpose(1, 0, 2).reshape(P, WPD * F).astype(BF16)
        )

        in_maps.append(
            {
                "t1": T1,
                "idx16": idx16,
                "dl128": dl128,
                "disb": disb,
                "xso": xso,
                "iota": iota_np,
                "ident": ident_np,
                "w1": w1b,
                "w2": w2b,
                "w3": w3b,
                "b1f": b1f,
                "b2f": b2f,
                "b3f": b3f,
            }
        )
    return meta, in_maps


def _build_program(meta):
    import os
    import concourse.bacc as bacc
    import concourse.mybir as mybir
    import concourse.tile as tile
    from contextlib import ExitStack

    dbg_layers = int(os.environ.get("GNN_LAYERS", "3"))
    dbg_bcap = int(os.environ.get("GNN_BATCH_CAP", str(NBATCH)))
    dbg_coll = os.environ.get("GNN_COLL", "1") == "1"
    nqueues = int(os.environ.get("GNN_QUEUES", "4"))
    ohb = int(os.environ.get("GNN_OHB", "16"))
    gchunk = int(os.environ.get("GNN_GCHUNK", "8"))
    scratch = int(os.environ.get("GNN_SCRATCH", "16384"))

    dt = mybir.dt
    nc = bacc.Bacc(
        "TRN2",
        target_bir_lowering=False,
        debug=False,
        num_devices=M,
        num_swdge_queues=nqueues,
        dynamic_dma_scratch_size=scratch,
    )

    t1 = nc.dram_tensor("t1", [TROWS, F], dt.bfloat16, kind="ExternalInput")
    idxd = nc.dram_tensor("idx16", [P, meta["sc16"]], dt.int16, kind="ExternalInput")
    dld = nc.dram_tensor("dl128", [P, meta["sccols"]], dt.bfloat16, kind="ExternalInput")
    disd = nc.dram_tensor("disb", [P, WPD], dt.float32, kind="ExternalInput")
    xsod = nc.dram_tensor("xso", [P, WPD * F], dt.bfloat16, kind="ExternalInput")
    iod = nc.dram_tensor("iota", [P, 1, P], dt.bfloat16, kind="ExternalInput")
    idnd = nc.dram_tensor("ident", [P, P], dt.bfloat16, kind="ExternalInput")
    w1d = nc.dram_tensor("w1", [F, F], dt.bfloat16, kind="ExternalInput")
    w2d = nc.dram_tensor("w2", [F, F], dt.bfloat16, kind="ExternalInput")
    w3d = nc.dram_tensor("w3", [F, DOUT], dt.bfloat16, kind="ExternalInput")
    b1d = nc.dram_tensor("b1f", [P, F], dt.float32, kind="ExternalInput")
    b2d = nc.dram_tensor("b2f", [P, F], dt.float32, kind="ExternalInput")
    b3d = nc.dram_tensor("b3f", [P, DOUT], dt.float32, kind="ExternalInput")
    outd = nc.dram_tensor("out", [BLKP, DOUT], dt.float32, kind="ExternalOutput")

    with tile.TileContext(nc) as tc, ExitStack() as ctx:
        const = ctx.enter_context(tc.tile_pool(name="const", bufs=1))
        dram = ctx.enter_context(tc.tile_pool(name="dram", bufs=1, space="DRAM"))
        ipool = ctx.enter_context(tc.tile_pool(name="ip", bufs=16))
        dpool = ctx.enter_context(tc.tile_pool(name="dp", bufs=16))
        gpool = ctx.enter_context(tc.tile_pool(name="gp", bufs=int(os.environ.get("GNN_GBUFS", "8"))))
        ohpool = ctx.enter_context(tc.tile_pool(name="ohp", bufs=int(os.environ.get("GNN_OHBUFS", "7"))))
        lhpool = ctx.enter_context(tc.tile_pool(name="lhp", bufs=3))
        zbpool = ctx.enter_context(tc.tile_pool(name="zbp", bufs=4))
        stage = ctx.enter_context(tc.tile_pool(name="stage", bufs=2))
        apsum = ctx.enter_context(tc.tile_pool(name="apsum", bufs=int(os.environ.get("GNN_APSUM", "4")), space="PSUM"))
        zpsum = ctx.enter_context(tc.tile_pool(name="zpsum", bufs=2, space="PSUM"))
        upool = ctx.enter_context(tc.tile_pool(name="up", bufs=3))
        tpsum = ctx.enter_context(tc.tile_pool(name="tpsum", bufs=2, space="PSUM"))

        # batch-0 idx/dl loads FIRST so gathers start before the const DMAs
        warm = {}
        for p in range(NPAIR):
            c = meta["calls"][(0, p)]
            it = ipool.tile([P, c.c16], dt.int16, tag="idx", name=f"wit_{p}")
            nc.sync.dma_start(out=it[:], in_=idxd[:, c.ic0 : c.ic0 + c.c16])
            dt_ = dpool.tile([P, c.ncols, 1], dt.bfloat16, tag="dl", name=f"wdl_{p}")
            nc.sync.dma_start(
                out=dt_[:],
                in_=dld[:, c.dc0 : c.dc0 + c.ncols].rearrange("p (c o) -> p c o", o=1),
            )
            warm[(0, p)] = (it, dt_)

        def cload(name, dram_t, shape, dtype, eng=None):
            tl = const.tile(shape, dtype, name=name)
            (eng or nc.sync).dma_start(out=tl[:], in_=dram_t[:])
            return tl

        iot = cload("iot", iod, [P, 1, P], dt.bfloat16)
        idn = cload("idn", idnd, [P, P], dt.bfloat16)
        dis_t = cload("dis_t", disd, [P, WPD], dt.float32)
        w1t = cload("w1t", w1d, [F, F], dt.bfloat16)
        w2t = cload("w2t", w2d, [F, F], dt.bfloat16)
        w3t = cload("w3t", w3d, [F, DOUT], dt.bfloat16)
        b1t = cload("b1t", b1d, [P, F], dt.float32)
        b2t = cload("b2t", b2d, [P, F], dt.float32)
        b3t = cload("b3t", b3d, [P, DOUT], dt.float32)

        tin2 = dram.tile([BLKP, F], dt.bfloat16, name="tin2")
        tin3 = dram.tile([BLKP, F], dt.bfloat16, name="tin3")
        tf2 = [
            dram.tile([REGR[c], F], dt.bfloat16, addr_space="Shared", name=f"tf2_{c}")
            for c in range(4)
        ]
        tf3 = [
            dram.tile([REGR[c], F], dt.bfloat16, addr_space="Shared", name=f"tf3_{c}")
            for c in range(4)
        ]

        calls = meta["calls"]
        gtot = meta["gtot"]

        qctr = [0]

        def do_layer(l, src_of, wt, bt, self_t, tst, tin=None, tfull=None, pending=None):
            gctr = [0] * WPD
            cache = {}
            nb = min(NBATCH, dbg_bcap)
            ret_pending = [None]
            agq = []

            def prep(wb, plist):
                tiles = {}
                for p in plist:
                    c = calls[(wb, p)]
                    if l == 0 and (wb, p) in warm:
                        it, dt_ = warm[(wb, p)]
                    else:
                        it = ipool.tile(
                            [P, c.c16], dt.int16, tag="idx", name=f"it{l}_{wb}_{p}"
                        )
                        nc.sync.dma_start(out=it[:], in_=idxd[:, c.ic0 : c.ic0 + c.c16])
                        dt_ = dpool.tile(
                            [P, c.ncols, 1], dt.bfloat16, tag="dl", name=f"dl{l}_{wb}_{p}"
                        )
                        nc.sync.dma_start(
                            out=dt_[:],
                            in_=dld[:, c.dc0 : c.dc0 + c.ncols].rearrange(
                                "p (c o) -> p c o", o=1
                            ),
                        )
                    gt = gpool.tile(
                        [P, c.c128, F], dt.bfloat16, tag="g", name=f"gt{l}_{wb}_{p}"
                    )
                    tiles[p] = (c, it, dt_, gt)
                # interleave gather chunks across pairs: window 0's matmuls
                # need the EARLY groups of all pairs, so issue those first
                ks = {p: 0 for p in plist}
                alive = list(plist)
                while alive:
                    for p in list(alive):
                        c, it, dt_, gt = tiles[p]
                        k0 = ks[p]
                        if k0 >= c.c128:
                            alive.remove(p)
                            continue
                        kc = min(gchunk, c.c128 - k0)
                        nc.gpsimd.dma_gather(
                            gt[:, k0 : k0 + kc, :],
                            src_of(p),
                            it[:, k0 * 8 : (k0 + kc) * 8],
                            kc * P,
                            kc * P,
                            F,
                            queue_num=qctr[0] % nqueues,
                        )
                        qctr[0] += 1
                        ks[p] += kc
                for p in plist:
                    c, it, dt_, gt = tiles[p]
                    oh = ohpool.tile(
                        [P, c.ncols, P], dt.bfloat16, tag="oh", name=f"oh{l}_{wb}_{p}"
                    )
                    for c0 in range(0, c.ncols, ohb):
                        cb = min(ohb, c.ncols - c0)
                        nc.vector.tensor_tensor(
                            out=oh[:, c0 : c0 + cb, :],
                            in0=dt_[:, c0 : c0 + cb, :].to_broadcast([P, cb, P]),
                            in1=iot[:].to_broadcast([P, cb, P]),
                            op=mybir.AluOpType.is_equal,
                        )
                    cache[(wb, p)] = (gt, oh)

            def handle_stage(wb):
                """Stage THIS batch's windows into tin right away (tin is
                local DRAM, multiple writers allowed); at a sub-chunk
                boundary emit the AllGather whose input is then ~ready."""
                if l == 2:
                    return
                bw0 = wb * WB
                bwc = min(WB, WPD - bw0)
                nc.scalar.dma_start(
                    out=tin[bw0 * P : (bw0 + bwc) * P, :].rearrange(
                        "(w p) f -> p w f", p=P
                    ),
                    in_=tst[:, bw0 * F : (bw0 + bwc) * F].rearrange(
                        "p (w f) -> p w f", f=F
                    ),
                )
                if (wb + 1) not in SUBB[1:]:
                    return
                s = SUBB.index(wb + 1) - 1
                fw = F
                r0, rn, w0, wn = SUBBASE[s], SUBROWS[s], SUBB[s] * WB, SUBW[s]
                r = REGOF[s]
                ob = SUBREGB[s]

                def emit_ag():
                    if dbg_coll:
                        nc.gpsimd.collective_compute(
                            "AllGather",
                            mybir.AluOpType.bypass,
                            replica_groups=[list(range(M))],
                            ins=[tin[r0 : r0 + rn, :].opt()],
                            outs=[tfull[r][ob : ob + M * rn, :].opt()],
                        )
                    else:
                        nc.sync.dma_start(
                            out=tfull[r][ob : ob + rn, :].rearrange(
                                "(w p) f -> p w f", p=P
                            ),
                            in_=tst[:, w0 * fw : (w0 + wn) * fw].rearrange(
                                "p (w f) -> p w f", f=fw
                            ),
                        )

                if s == NSUB - 1:
                    ret_pending[0] = emit_ag
                else:
                    emit_ag()

            def windows(wb):
                w0 = wb * WB
                wcnt = min(WB, WPD - w0)
                gts = [cache[(wb, p)][0] for p in range(NPAIR)]
                ohs = [cache[(wb, p)][1] for p in range(NPAIR)]
                for w in range(w0, w0 + wcnt):
                    agg = apsum.tile([P, P], dt.float32, tag="agg", name=f"agg{l}_{w}")
                    # self-loop contribution via identity: agg = I^T @ T_l[own rows]
                    nc.tensor.matmul(
                        agg[:],
                        lhsT=idn[:],
                        rhs=self_t[:, w * F : (w + 1) * F],
                        start=True,
                        stop=False,
                    )
                    for p in range(NPAIR):
                        c = calls[(wb, p)]
                        for g, col in c.mms[w]:
                            gctr[w] += 1
                            sp = gctr[w] == gtot[w]
                            nc.tensor.matmul(
                                agg[:],
                                lhsT=ohs[p][:, col, :],
                                rhs=gts[p][:, g, :],
                                start=False,
                                stop=sp,
                            )
                    u = upool.tile([P, P], dt.bfloat16, tag="u", name=f"u{l}_{w}")
                    nc.vector.tensor_scalar(
                        u[:], agg[:], dis_t[:, w : w + 1], None, mybir.AluOpType.mult
                    )
                    tp = tpsum.tile([P, P], dt.bfloat16, tag="tp", name=f"tp{l}_{w}")
                    nc.tensor.transpose(tp[:], u[:], idn[:])
                    lh = lhpool.tile([P, P], dt.bfloat16, tag="lh", name=f"lh{l}_{w}")
                    nc.vector.tensor_copy(out=lh[:], in_=tp[:])
                    zw = zpsum.tile(
                        [P, F if l < 2 else DOUT], dt.float32, tag="zp", name=f"z{l}_{w}"
                    )
                    nc.tensor.matmul(zw[:], lhsT=lh[:], rhs=wt[:], start=True, stop=True)
                    if l < 2:
                        zc = zbpool.tile([P, F], dt.float32, tag="zb", name=f"zc{l}_{w}")
                        nc.vector.tensor_tensor(
                            out=zc[:], in0=zw[:], in1=bt[:], op=mybir.AluOpType.add
                        )
                        nc.scalar.activation(
                            tst[:, w * F : (w + 1) * F],
                            zc[:],
                            mybir.ActivationFunctionType.Relu,
                            scale=dis_t[:, w : w + 1],
                        )
                    else:
                        nc.vector.tensor_tensor(
                            out=tst[:, w * DOUT : (w + 1) * DOUT],
                            in0=zw[:],
                            in1=bt[:],
                            op=mybir.AluOpType.add,
                        )
                if l == 2:
                    nc.sync.dma_start(
                        out=outd[w0 * P : (w0 + wcnt) * P, :].rearrange(
                            "(w p) f -> p w f", p=P
                        ),
                        in_=tst[:, w0 * DOUT : (w0 + wcnt) * DOUT].rearrange(
                            "p (w f) -> p w f", f=DOUT
                        ),
                    )

            defer = int(os.environ.get('GNN_DEFER', '0')) if (pending is not None and nb > 2) else 0
            for wb in range(defer):
                prep(wb, [0, 1, 2])
            if pending is not None:
                pending()
            for wb in range(defer):
                prep(wb, [3])
                windows(wb)
                handle_stage(wb)
            for wb in range(defer, nb):
                while agq and agq[0][1] <= wb:
                    agq.pop(0)[0]()
                prep(wb, [0, 1, 2, 3])
                windows(wb)
                handle_stage(wb)
            for fn, _ in agq:
                fn()
            return ret_pending[0]

        def reg(t):
            if isinstance(t, list):
                return lambda p: t[p][:]
            return lambda p: t[REGB[p] : REGB[p] + REGR[p], :]

        xso_t = stage.tile([P, WPD * F], dt.bfloat16, tag="tstage", name="xso_t")
        nc.scalar.dma_start(out=xso_t[:], in_=xsod[:])
        ts1 = stage.tile([P, WPD * F], dt.bfloat16, tag="tstage", name="ts1")
        pend = do_layer(0, reg(t1), w1t, b1t, xso_t, ts1, tin2, tf2)
        if dbg_layers >= 2:
            ts2 = stage.tile([P, WPD * F], dt.bfloat16, tag="tstage", name="ts2")
            pend = do_layer(1, reg(tf2), w2t, b2t, ts1, ts2, tin3, tf3, pending=pend)
        if dbg_layers >= 3:
            ts3 = stage.tile([P, WPD * DOUT], dt.float32, tag="tstage", name="ts3")
            do_layer(2, reg(tf3), w3t, b3t, ts2, ts3, pending=pend)
        else:
            zts = stage.tile([P, WPD * DOUT], dt.float32, tag="tstage", name="zts")
            nc.vector.memset(zts[:], 0.0)
            nc.sync.dma_start(
                out=outd[:].rearrange("(w p) f -> p w f", p=P),
                in_=zts[:].rearrange("p (w f) -> p w f", f=DOUT),
            )

    nc.compile()
    return nc


_CACHE = {}


def _get_program(meta):
    import os

    key = (
        meta["sc16"],
        meta["sccols"],
        os.environ.get("GNN_LAYERS"),
        os.environ.get("GNN_BATCH_CAP"),
        os.environ.get("GNN_COLL"),
        os.environ.get("GNN_QUEUES"),
        os.environ.get("GNN_OHB"),
        os.environ.get("GNN_GCHUNK"),
        os.environ.get("GNN_SCRATCH"),
        os.environ.get("GNN_SUBB"),
        os.environ.get("GNN_WB"),
        os.environ.get("GNN_GBUFS"),
        os.environ.get("GNN_OHBUFS"),
        os.environ.get("GNN_DEFER"),
        os.environ.get("GNN_APSUM"),
    )
    if key not in _CACHE:
        _CACHE[key] = _build_program(meta)
    return _CACHE[key]


def run(trace=False, **inputs):
    from concourse.bass_utils import run_bass_kernel_spmd

    meta, in_maps = _preprocess(**inputs)
    nc = _get_program(meta)
    res = run_bass_kernel_spmd(nc, in_maps, core_ids=list(range(M)), trace=trace)
    out = np.empty((N, DOUT), np.float32)
    for i in range(M):
        out[i * BLK : (i + 1) * BLK] = res.results[i]["out"][:BLK]
    return out, res


def kernel(**inputs):
    out, _ = run(trace=False, **inputs)
    return out


# revision 36
# speedup vs baseline: 1.1500x; 1.1500x over previous
"""3-layer GCN (ContrastiveGNN) on 8 Trainium2 NeuronCores.

Strategy (dst-sharded edge partition, "1D graph partition"):
  - Nodes are split into 8 blocks of 12500 dsts; device i owns block i and all
    edges whose dst lands in its block.  Self-loops are NOT gathered: their
    contribution dis[d]^2*h[d] is added locally from the previous layer's
    stage tile (layer 1 uses a per-device staged copy of dis*x).
  - Math reorder: for each GCN layer,
        out = D^-1/2 (A+I) D^-1/2 (h W) + b  ==  dis_d * (sum_{e->d} T[src]) @ W + b
    with T = dis * h (row-scaled activations).  Aggregation happens BEFORE the
    dense transform, so the gather tables carry 128 features for every layer.
  - Aggregation on the tensor engine: edges are DENSELY packed per
    (window-batch, src-region) call, sorted by dst-window; each 128-slot
    gather group feeds one_hot[slot, dst_local].T @ gathered[slot, feat]
    matmuls accumulated in PSUM per 128-dst window.  A group may straddle
    window boundaries: for each (group, window) pair that is non-empty on ANY
    device there is one static matmul + one one-hot column; per-device
    one-hot entries are -1 (-> all-zero column) for slots outside the window,
    so the SPMD-uniform instruction stream stays correct per device.
  - Gathers use the SWDGE dma_gather custom instruction (int16 indices =>
    the 100352-row table is addressed in 4 chunk regions of <=32768 rows).
  - Tables are bf16 (PSUM accumulation f32); between layers the per-device
    table blocks are exchanged with chunked AllGather collectives overlapped
    with the remaining windows' compute.
  - All 8 devices run one SPMD program: per-call slot capacities and
    (group, window) matmul lists are maxed/unioned over devices, so
    instruction streams are identical and only input data differs.
"""

import numpy as np
import ml_dtypes

BF16 = ml_dtypes.bfloat16

N = 100000
F = 128
DOUT = 64
M = 8
BLK = N // M            # 12500 dst nodes per device
P = 128
WPD = (BLK + P - 1) // P  # 98 windows per device
BLKP = WPD * P            # 12544 padded block rows
TROWS = M * BLKP          # 100352 table rows
NPAIR = 4
import os as _os
WB = int(_os.environ.get("GNN_WB", "6"))  # windows per gather batch
NBATCH = (WPD + WB - 1) // WB

# AG sub-chunks (staging/collective units) and gather regions (int16
# addressing units).  Sub-chunk s covers batches SUBB[s]..SUBB[s+1] and
# belongs to region REGOF[s]; regions are the 4 contiguous table areas a
# gather call can address; within a region rows are sub-chunk-major.
SUBB = [int(b) for b in _os.environ.get("GNN_SUBB", "0,5,10,15,17" if WB == 6 else "").split(",")]
NSUB = len(SUBB) - 1
REGOF = [min(s, 3) for s in range(NSUB)]        # subs >=3 share region 3
SUBW = [min(SUBB[s + 1] * WB, WPD) - SUBB[s] * WB for s in range(NSUB)]
SUBBASE = [SUBB[s] * WB * P for s in range(NSUB)]  # local row base of sub
SUBROWS = [w * P for w in SUBW]                 # local rows per sub
# region-local row base of each sub (sub-chunk-major within region)
SUBREGB = []
_acc = {}
for _s in range(NSUB):
    _r = REGOF[_s]
    SUBREGB.append(_acc.get(_r, 0))
    _acc[_r] = _acc.get(_r, 0) + M * SUBROWS[_s]
REGR = [_acc.get(r, 0) for r in range(4)]       # table rows per region
REGB = [0, 0, 0, 0]
for _r in range(1, 4):
    REGB[_r] = REGB[_r - 1] + REGR[_r - 1]      # table row base of region
assert all(r <= 32768 for r in REGR), REGR


class _Call:
    __slots__ = ("ic0", "c16", "dc0", "ncols", "c128", "nslots", "mms")


def _preprocess(x, edge_index, W1, b1, W2, b2, W3, b3):
    """Host-side index plumbing + input staging. Returns (meta, per-core
    in_maps)."""
    x = np.asarray(x, np.float32)
    ei = np.asarray(edge_index)
    src = ei[0].astype(np.int64)
    dst = ei[1].astype(np.int64)

    # degree/dis INCLUDE self-loops (reference adds them), but self-loop
    # edges are handled locally, not gathered.
    deg = (np.bincount(dst, minlength=N) + 1).astype(np.float32)
    dis = (1.0 / np.sqrt(deg)).astype(np.float32)

    # layer-1 gather table: dis-scaled input features, zeroed pad rows,
    # sub-chunk-major layout: sub s = [M blocks x SUBROWS[s] local rows]
    xs = x * dis[:, None]
    T1 = np.zeros((TROWS, F), BF16)
    for s in range(NSUB):
        for j in range(M):
            lo = j * BLK + SUBBASE[s]
            hi = min(j * BLK + SUBBASE[s] + SUBROWS[s], (j + 1) * BLK)
            db = REGB[REGOF[s]] + SUBREGB[s] + j * SUBROWS[s]
            T1[db : db + (hi - lo)] = xs[lo:hi].astype(BF16)

    dev = dst // BLK
    j_src = src // BLK
    loc_src = src - j_src * BLK
    # src sub-chunk id -> region + region-local row
    s_src = np.searchsorted(np.array(SUBBASE[1:], np.int64), loc_src, side="right")
    subrows = np.array(SUBROWS, np.int64)
    subbase = np.array(SUBBASE, np.int64)
    subregb = np.array(SUBREGB, np.int64)
    regof = np.array(REGOF, np.int64)
    c_src = regof[s_src]
    rel = (
        subregb[s_src] + j_src * subrows[s_src] + (loc_src - subbase[s_src])
    ).astype(np.int64)
    dloc = dst - dev * BLK
    w_arr = dloc // P
    dwin = (dloc - w_arr * P).astype(np.int64)
    wb_arr = w_arr // WB
    # bucket key: (batch, region, window) — buckets of one call are contiguous
    bkey = (wb_arr * NPAIR + c_src) * WPD + w_arr
    NBUCK = NBATCH * NPAIR * WPD

    cnt = np.zeros((M, NBUCK), np.int64)
    for i in range(M):
        cnt[i] = np.bincount(bkey[dev == i], minlength=NBUCK)

    # window start/end slot positions per device within each call
    # (dense packing: edges of a call sorted by window, tightly packed)
    meta_calls = {}
    gtot = np.zeros(WPD, np.int64)
    ic = dc = 0
    for wb in range(NBATCH):
        w0 = wb * WB
        wcnt = min(WB, WPD - w0)
        for p in range(NPAIR):
            c = _Call()
            c.ic0, c.dc0 = ic, dc
            bids = [(wb * NPAIR + p) * WPD + w for w in range(w0, w0 + wcnt)]
            ccnt = cnt[:, bids]                       # [M, wcnt]
            ends = np.cumsum(ccnt, axis=1)            # [M, wcnt]
            starts = ends - ccnt
            nmax = int(ends[:, -1].max())
            nslots = -(-max(nmax, 1) // P) * P
            c.nslots = nslots
            c.c16 = nslots // 16
            c.c128 = nslots // P
            c.mms = {}
            ncols = 0
            for k, w in enumerate(range(w0, w0 + wcnt)):
                has = ccnt[:, k] > 0
                if not has.any():
                    c.mms[w] = []
                    continue
                g_lo = int(starts[has, k].min()) // P
                g_hi = -(-int(ends[has, k].max()) // P)
                c.mms[w] = [(g, ncols + j) for j, g in enumerate(range(g_lo, g_hi))]
                ncols += g_hi - g_lo
                gtot[w] += g_hi - g_lo
            c.ncols = ncols
            ic += c.c16
            dc += c.ncols
            meta_calls[(wb, p)] = c
    sc16, sccols = ic, dc

    meta = {"calls": meta_calls, "gtot": gtot, "sc16": sc16, "sccols": sccols}

    iota_np = np.tile(np.arange(P, dtype=np.float32).astype(BF16), (P, 1)).reshape(
        P, 1, P
    )
    ident_np = np.eye(P, dtype=np.float32).astype(BF16)
    w1b = np.asarray(W1, np.float32).astype(BF16)
    w2b = np.asarray(W2, np.float32).astype(BF16)
    w3b = np.asarray(W3, np.float32).astype(BF16)
    b1f = np.tile(np.asarray(b1, np.float32), (P, 1))
    b2f = np.tile(np.asarray(b2, np.float32), (P, 1))
    b3f = np.tile(np.asarray(b3, np.float32), (P, 1))

    in_maps = []
    for i in range(M):
        m = dev == i
        bk = bkey[m]
        o = np.argsort(bk, kind="stable")
        bk_s = bk[o]
        rel_s = rel[m][o].astype(np.int16)
        dw_s = dwin[m][o]

        i16_parts, dl_parts = [], []
        pos = 0
        for wb in range(NBATCH):
            w0 = wb * WB
            wcnt = min(WB, WPD - w0)
            for p in range(NPAIR):
                c = meta_calls[(wb, p)]
                bids = [(wb * NPAIR + p) * WPD + w for w in range(w0, w0 + wcnt)]
                ccnt_i = cnt[i][bids]
                n_i = int(ccnt_i.sum())
                seg_r = rel_s[pos : pos + n_i]
                seg_w = dw_s[pos : pos + n_i]
                pos += n_i
                idxfl = np.zeros(c.nslots, np.int16)
                idxfl[:n_i] = seg_r
                dl = np.full((P, c.ncols), -1.0, np.float32)
                ends_i = np.cumsum(ccnt_i)
                starts_i = ends_i - ccnt_i
                for k, w in enumerate(range(w0, w0 + wcnt)):
                    st, en = int(starts_i[k]), int(ends_i[k])
                    if st == en:
                        continue
                    for g, col in c.mms[w]:
                        lo = max(st, g * P)
                        hi = min(en, (g + 1) * P)
                        if lo < hi:
                            dl[lo - g * P : hi - g * P, col] = seg_w[lo:hi]
                i16_parts.append(idxfl.reshape(-1, 16).T)
                dl_parts.append(dl)
        idx16 = np.tile(np.concatenate(i16_parts, axis=1), (8, 1))
        dl128 = np.concatenate(dl_parts, axis=1).astype(BF16)

        disp = np.zeros(BLKP, np.float32)
        disp[:BLK] = dis[i * BLK : (i + 1) * BLK]
        disb = disp.reshape(WPD, P).T.copy()

        # own-block layer-1 table rows in stage layout [P, WPD*F]
        xsp = np.zeros((BLKP, F), np.float32)
        xsp[:BLK] = xs[i * BLK : (i + 1) * BLK]
        xso = (
            xsp.reshape(WPD, P, F).transpose(1, 0, 2).reshape(P, WPD * F).astype(BF16)
        )

        in_maps.append(
            {
                "t1": T1,
                "idx16": idx16,
                "dl128": dl128,
                "disb": disb,
                "xso": xso,
                "iota": iota_np,
                "ident": ident_np,
                "w1": w1b,
                "w2": w2b,
                "w3": w3b,
                "b1f": b1f,
                "b2f": b2f,
                "b3f": b3f,
            }
        )
    return meta, in_maps


def _build_program(meta):
    import os
    import concourse.bacc as bacc
    import concourse.mybir as mybir
    import concourse.tile as tile
    from contextlib import ExitStack

    dbg_layers = int(os.environ.get("GNN_LAYERS", "3"))
    dbg_bcap = int(os.environ.get("GNN_BATCH_CAP", str(NBATCH)))
    dbg_coll = os.environ.get("GNN_COLL", "1") == "1"
    nqueues = int(os.environ.get("GNN_QUEUES", "4"))
    ohb = int(os.environ.get("GNN_OHB", "16"))
    gchunk = int(os.environ.get("GNN_GCHUNK", "8"))
    scratch = int(os.environ.get("GNN_SCRATCH", "16384"))

    dt = mybir.dt
    nc = bacc.Bacc(
        "TRN2",
        target_bir_lowering=False,
        debug=False,
        num_devices=M,
        num_swdge_queues=nqueues,
        dynamic_dma_scratch_size=scratch,
    )

    t1 = nc.dram_tensor("t1", [TROWS, F], dt.bfloat16, kind="ExternalInput")
    idxd = nc.dram_tensor("idx16", [P, meta["sc16"]], dt.int16, kind="ExternalInput")
    dld = nc.dram_tensor("dl128", [P, meta["sccols"]], dt.bfloat16, kind="ExternalInput")
    disd = nc.dram_tensor("disb", [P, WPD], dt.float32, kind="ExternalInput")
    xsod = nc.dram_tensor("xso", [P, WPD * F], dt.bfloat16, kind="ExternalInput")
    iod = nc.dram_tensor("iota", [P, 1, P], dt.bfloat16, kind="ExternalInput")
    idnd = nc.dram_tensor("ident", [P, P], dt.bfloat16, kind="ExternalInput")
    w1d = nc.dram_tensor("w1", [F, F], dt.bfloat16, kind="ExternalInput")
    w2d = nc.dram_tensor("w2", [F, F], dt.bfloat16, kind="ExternalInput")
    w3d = nc.dram_tensor("w3", [F, DOUT], dt.bfloat16, kind="ExternalInput")
    b1d = nc.dram_tensor("b1f", [P, F], dt.float32, kind="ExternalInput")
    b2d = nc.dram_tensor("b2f", [P, F], dt.float32, kind="ExternalInput")
    b3d = nc.dram_tensor("b3f", [P, DOUT], dt.float32, kind="ExternalInput")
    outd = nc.dram_tensor("out", [BLKP, DOUT], dt.float32, kind="ExternalOutput")

    with tile.TileContext(nc) as tc, ExitStack() as ctx:
        const = ctx.enter_context(tc.tile_pool(name="const", bufs=1))
        dram = ctx.enter_context(tc.tile_pool(name="dram", bufs=1, space="DRAM"))
        ipool = ctx.enter_context(tc.tile_pool(name="ip", bufs=16))
        dpool = ctx.enter_context(tc.tile_pool(name="dp", bufs=16))
        gpool = ctx.enter_context(tc.tile_pool(name="gp", bufs=int(os.environ.get("GNN_GBUFS", "8"))))
        ohpool = ctx.enter_context(tc.tile_pool(name="ohp", bufs=int(os.environ.get("GNN_OHBUFS", "7"))))
        lhpool = ctx.enter_context(tc.tile_pool(name="lhp", bufs=3))
        zbpool = ctx.enter_context(tc.tile_pool(name="zbp", bufs=4))
        stage = ctx.enter_context(tc.tile_pool(name="stage", bufs=2))
        apsum = ctx.enter_context(tc.tile_pool(name="apsum", bufs=int(os.environ.get("GNN_APSUM", "4")), space="PSUM"))
        zpsum = ctx.enter_context(tc.tile_pool(name="zpsum", bufs=2, space="PSUM"))
        upool = ctx.enter_context(tc.tile_pool(name="up", bufs=3))
        tpsum = ctx.enter_context(tc.tile_pool(name="tpsum", bufs=2, space="PSUM"))

        def cload(name, dram_t, shape, dtype, eng=None):
            tl = const.tile(shape, dtype, name=name)
            (eng or nc.sync).dma_start(out=tl[:], in_=dram_t[:])
            return tl

        iot = cload("iot", iod, [P, 1, P], dt.bfloat16)
        idn = cload("idn", idnd, [P, P], dt.bfloat16)
        dis_t = cload("dis_t", disd, [P, WPD], dt.float32)
        w1t = cload("w1t", w1d, [F, F], dt.bfloat16)
        w2t = cload("w2t", w2d, [F, F], dt.bfloat16)
        w3t = cload("w3t", w3d, [F, DOUT], dt.bfloat16)
        b1t = cload("b1t", b1d, [P, F], dt.float32)
        b2t = cload("b2t", b2d, [P, F], dt.float32)
        b3t = cload("b3t", b3d, [P, DOUT], dt.float32)

        tin2 = dram.tile([BLKP, F], dt.bfloat16, name="tin2")
        tin3 = dram.tile([BLKP, F], dt.bfloat16, name="tin3")
        tf2 = [
            dram.tile([REGR[c], F], dt.bfloat16, addr_space="Shared", name=f"tf2_{c}")
            for c in range(4)
        ]
        tf3 = [
            dram.tile([REGR[c], F], dt.bfloat16, addr_space="Shared", name=f"tf3_{c}")
            for c in range(4)
        ]

        calls = meta["calls"]
        gtot = meta["gtot"]

        qctr = [0]

        def do_layer(l, src_of, wt, bt, self_t, tst, tin=None, tfull=None, pending=None):
            gctr = [0] * WPD
            cache = {}
            nb = min(NBATCH, dbg_bcap)
            ret_pending = [None]
            agq = []

            def prep(wb, plist):
                tiles = {}
                for p in plist:
                    c = calls[(wb, p)]
                    it = ipool.tile([P, c.c16], dt.int16, tag="idx", name=f"it{l}_{wb}_{p}")
                    nc.sync.dma_start(out=it[:], in_=idxd[:, c.ic0 : c.ic0 + c.c16])
                    dt_ = dpool.tile(
                        [P, c.ncols, 1], dt.bfloat16, tag="dl", name=f"dl{l}_{wb}_{p}"
                    )
                    nc.sync.dma_start(
                        out=dt_[:],
                        in_=dld[:, c.dc0 : c.dc0 + c.ncols].rearrange(
                            "p (c o) -> p c o", o=1
                        ),
                    )
                    gt = gpool.tile(
                        [P, c.c128, F], dt.bfloat16, tag="g", name=f"gt{l}_{wb}_{p}"
                    )
                    tiles[p] = (c, it, dt_, gt)
                # interleave gather chunks across pairs: window 0's matmuls
                # need the EARLY groups of all pairs, so issue those first
                ks = {p: 0 for p in plist}
                alive = list(plist)
                while alive:
                    for p in list(alive):
                        c, it, dt_, gt = tiles[p]
                        k0 = ks[p]
                        if k0 >= c.c128:
                            alive.remove(p)
                            continue
                        kc = min(gchunk, c.c128 - k0)
                        nc.gpsimd.dma_gather(
                            gt[:, k0 : k0 + kc, :],
                            src_of(p),
                            it[:, k0 * 8 : (k0 + kc) * 8],
                            kc * P,
                            kc * P,
                            F,
                            queue_num=qctr[0] % nqueues,
                        )
                        qctr[0] += 1
                        ks[p] += kc
                for p in plist:
                    c, it, dt_, gt = tiles[p]
                    oh = ohpool.tile(
                        [P, c.ncols, P], dt.bfloat16, tag="oh", name=f"oh{l}_{wb}_{p}"
                    )
                    for c0 in range(0, c.ncols, ohb):
                        cb = min(ohb, c.ncols - c0)
                        nc.vector.tensor_tensor(
                            out=oh[:, c0 : c0 + cb, :],
                            in0=dt_[:, c0 : c0 + cb, :].to_broadcast([P, cb, P]),
                            in1=iot[:].to_broadcast([P, cb, P]),
                            op=mybir.AluOpType.is_equal,
                        )
                    cache[(wb, p)] = (gt, oh)

            def handle_stage(wb):
                """Stage THIS batch's windows into tin right away (tin is
                local DRAM, multiple writers allowed); at a sub-chunk
                boundary emit the AllGather whose input is then ~ready."""
                if l == 2:
                    return
                bw0 = wb * WB
                bwc = min(WB, WPD - bw0)
                nc.scalar.dma_start(
                    out=tin[bw0 * P : (bw0 + bwc) * P, :].rearrange(
                        "(w p) f -> p w f", p=P
                    ),
                    in_=tst[:, bw0 * F : (bw0 + bwc) * F].rearrange(
                        "p (w f) -> p w f", f=F
                    ),
                )
                if (wb + 1) not in SUBB[1:]:
                    return
                s = SUBB.index(wb + 1) - 1
                fw = F
                r0, rn, w0, wn = SUBBASE[s], SUBROWS[s], SUBB[s] * WB, SUBW[s]
                r = REGOF[s]
                ob = SUBREGB[s]

                def emit_ag():
                    if dbg_coll:
                        nc.gpsimd.collective_compute(
                            "AllGather",
                            mybir.AluOpType.bypass,
                            replica_groups=[list(range(M))],
                            ins=[tin[r0 : r0 + rn, :].opt()],
                            outs=[tfull[r][ob : ob + M * rn, :].opt()],
                        )
                    else:
                        nc.sync.dma_start(
                            out=tfull[r][ob : ob + rn, :].rearrange(
                                "(w p) f -> p w f", p=P
                            ),
                            in_=tst[:, w0 * fw : (w0 + wn) * fw].rearrange(
                                "p (w f) -> p w f", f=fw
                            ),
                        )

                if s == NSUB - 1:
                    ret_pending[0] = emit_ag
                else:
                    emit_ag()

            def windows(wb):
                w0 = wb * WB
                wcnt = min(WB, WPD - w0)
                gts = [cache[(wb, p)][0] for p in range(NPAIR)]
                ohs = [cache[(wb, p)][1] for p in range(NPAIR)]
                for w in range(w0, w0 + wcnt):
                    agg = apsum.tile([P, P], dt.float32, tag="agg", name=f"agg{l}_{w}")
                    # self-loop contribution via identity: agg = I^T @ T_l[own rows]
                    nc.tensor.matmul(
                        agg[:],
                        lhsT=idn[:],
                        rhs=self_t[:, w * F : (w + 1) * F],
                        start=True,
                        stop=False,
                    )
                    for p in range(NPAIR):
                        c = calls[(wb, p)]
                        for g, col in c.mms[w]:
                            gctr[w] += 1
                            sp = gctr[w] == gtot[w]
                            nc.tensor.matmul(
                                agg[:],
                                lhsT=ohs[p][:, col, :],
                                rhs=gts[p][:, g, :],
                                start=False,
                                stop=sp,
                            )
                    u = upool.tile([P, P], dt.bfloat16, tag="u", name=f"u{l}_{w}")
                    nc.vector.tensor_scalar(
                        u[:], agg[:], dis_t[:, w : w + 1], None, mybir.AluOpType.mult
                    )
                    tp = tpsum.tile([P, P], dt.bfloat16, tag="tp", name=f"tp{l}_{w}")
                    nc.tensor.transpose(tp[:], u[:], idn[:])
                    lh = lhpool.tile([P, P], dt.bfloat16, tag="lh", name=f"lh{l}_{w}")
                    nc.vector.tensor_copy(out=lh[:], in_=tp[:])
                    zw = zpsum.tile(
                        [P, F if l < 2 else DOUT], dt.float32, tag="zp", name=f"z{l}_{w}"
                    )
                    nc.tensor.matmul(zw[:], lhsT=lh[:], rhs=wt[:], start=True, stop=True)
                    if l < 2:
                        zc = zbpool.tile([P, F], dt.float32, tag="zb", name=f"zc{l}_{w}")
                        nc.vector.tensor_tensor(
                            out=zc[:], in0=zw[:], in1=bt[:], op=mybir.AluOpType.add
                        )
                        nc.scalar.activation(
                            tst[:, w * F : (w + 1) * F],
                            zc[:],
                            mybir.ActivationFunctionType.Relu,
                            scale=dis_t[:, w : w + 1],
                        )
                    else:
                        nc.vector.tensor_tensor(
                            out=tst[:, w * DOUT : (w + 1) * DOUT],
                            in0=zw[:],
                            in1=bt[:],
                            op=mybir.AluOpType.add,
                        )
                if l == 2:
                    nc.sync.dma_start(
                        out=outd[w0 * P : (w0 + wcnt) * P, :].rearrange(
                            "(w p) f -> p w f", p=P
                        ),
                        in_=tst[:, w0 * DOUT : (w0 + wcnt) * DOUT].rearrange(
                            "p (w f) -> p w f", f=DOUT
                        ),
                    )

            defer = int(os.environ.get('GNN_DEFER', '0')) if (pending is not None and nb > 2) else 0
            for wb in range(defer):
                prep(wb, [0, 1, 2])
            if pending is not None:
                pending()
            for wb in range(defer):
                prep(wb, [3])
                windows(wb)
                handle_stage(wb)
            for wb in range(defer, nb):
                while agq and agq[0][1] <= wb:
                    agq.pop(0)[0]()
                prep(wb, [0, 1, 2, 3])
                windows(wb)
                handle_stage(wb)
            for fn, _ in agq:
                fn()
            return ret_pending[0]

        def reg(t):
            if isinstance(t, list):
                return lambda p: t[p][:]
            return lambda p: t[REGB[p] : REGB[p] + REGR[p], :]

        xso_t = stage.tile([P, WPD * F], dt.bfloat16, tag="tstage", name="xso_t")
        nc.scalar.dma_start(out=xso_t[:], in_=xsod[:])
        ts1 = stage.tile([P, WPD * F], dt.bfloat16, tag="tstage", name="ts1")
        pend = do_layer(0, reg(t1), w1t, b1t, xso_t, ts1, tin2, tf2)
        if dbg_layers >= 2:
            ts2 = stage.tile([P, WPD * F], dt.bfloat16, tag="tstage", name="ts2")
            pend = do_layer(1, reg(tf2), w2t, b2t, ts1, ts2, tin3, tf3, pending=pend)
        if dbg_layers >= 3:
            ts3 = stage.tile([P, WPD * DOUT], dt.float32, tag="tstage", name="ts3")
            do_layer(2, reg(tf3), w3t, b3t, ts2, ts3, pending=pend)
        else:
            zts = stage.tile([P, WPD * DOUT], dt.float32, tag="tstage", name="zts")
            nc.vector.memset(zts[:], 0.0)
            nc.sync.dma_start(
                out=outd[:].rearrange("(w p) f -> p w f", p=P),
                in_=zts[:].rearrange("p (w f) -> p w f", f=DOUT),
            )

    nc.compile()
    return nc


_CACHE = {}


def _get_program(meta):
    import os

    key = (
        meta["sc16"],
        meta["sccols"],
        os.environ.get("GNN_LAYERS"),
        os.environ.get("GNN_BATCH_CAP"),
        os.environ.get("GNN_COLL"),
        os.environ.get("GNN_QUEUES"),
        os.environ.get("GNN_OHB"),
        os.environ.get("GNN_GCHUNK"),
        os.environ.get("GNN_SCRATCH"),
        os.environ.get("GNN_SUBB"),
        os.environ.get("GNN_WB"),
        os.environ.get("GNN_GBUFS"),
        os.environ.get("GNN_OHBUFS"),
        os.environ.get("GNN_DEFER"),
        os.environ.get("GNN_APSUM"),
    )
    if key not in _CACHE:
        _CACHE[key] = _build_program(meta)
    return _CACHE[key]


def run(trace=False, **inputs):
    from concourse.bass_utils import run_bass_kernel_spmd

    meta, in_maps = _preprocess(**inputs)
    nc = _get_program(meta)
    res = run_bass_kernel_spmd(nc, in_maps, core_ids=list(range(M)), trace=trace)
    out = np.empty((N, DOUT), np.float32)
    for i in range(M):
        out[i * BLK : (i + 1) * BLK] = res.results[i]["out"][:BLK]
    return out, res


def kernel(**inputs):
    out, _ = run(trace=False, **inputs)
    return out
